# revision 26
# baseline (speedup 1.0000x reference)
"""Trainium2 Bass kernel for nn_Block_47545287967557 (dense_cnn).

The reference module, simplified:
  - dead avgpool->linear->relu path (result unused)
  - sum over K=4 conv branches == ONE 3x3 VALID conv with weights Wc.sum(0)
    and bias bc.sum(0):  O[b,co,y,x] = sum_{ci,dy,dx} Weff[co,ci,dy,dx] *
    X[b,ci,y+dy,x+dx] + beff[co]
  X: [32,3,512,512] fp32 -> O: [32,3,510,510] fp32.

Strategy: pure data-parallel over batch across 8 NeuronCores (4 images each).
Per core the conv runs on the tensor engine as block-banded matmuls:
  contraction K = (c_in, yi) packed into 126 partitions (42-row y window)
  plus a constant-ones row 126 that carries the bias (stationary row 126 of
  the dx=0 matrix holds beff, so PSUM comes out pre-biased), output
  M = (c_out, yo) packed into 120 partitions (+8 zero pad to 128 for FWL),
  moving N = 510 x positions; one matmul per dx shift (3, PSUM-accumulated).
  13 y-blocks per image (y0 = 0,40,...,440,470; the last overlaps rows
  470..479 with identical values).

Precision/bandwidth: X is cast to fp16 on the HOST and DMA'd as fp16 (the
matmul runs in fp16; this halves input HBM traffic). fp8 was measured (CPU
sim, exact seed-0 data): e4m3 rel err 4.0e-2, e3m4 2.97e-2 -- both over the
2e-2 gate, so fp16 it is (3.4e-4). The PSUM->SBUF downcast copy alternates
between the scalar and vector engines (a single engine would serialize at
~35us). Output is stored fp16 (host upcasts to fp32 while unsharding).

DMA: trn2 has two HWDGE rings, FIFO per ring (sync/SP and scalar/Act).
Inputs stream on the SP ring, outputs + consts on the Act ring so reads and
writes overlap. Outputs of earlier images must NOT use the SP ring: a
sequencer issues in program order, so an output trigger there would block
later input triggers behind the compute dependency. Only the last image's
output is split across both rings (4 chunks) to halve the drain tail.
The stationary matrices load as ONE DMA [127, 3*128] (127 x 768B
descriptors, ~2.4us) -- as 3+1 separate tensors they were 555 x 256B
descriptors taking ~10us, stalling the first image's dx=1/2 matmuls.
"""

import sys

sys.path.insert(0, "/opt/trn_rl_repo")

import numpy as np

N_CORES = 8
B_PER_CORE = 4
C = 3
H = W = 512
OH = OW = 510
NBLK = 13
KP = C * 42 + 1  # 126 contraction partitions + ones row (bias)
MP = C * 40      # 120 live output partitions
MPAD = 128       # stationary columns padded for FWL
IN_CHUNKS = {
    0: [(0, 2), (2, 7), (7, 13)],   # small first chunk -> compute starts early
    1: [(0, 13)], 2: [(0, 13)], 3: [(0, 13)],  # whole-image DMAs, 13KB descs
}
OUT_CHUNKS = {
    0: [(0, 7), (7, 13)],
    1: [(0, 7), (7, 13)],
    2: [(0, 7), (7, 13)],
    3: [(0, 4), (4, 7), (7, 9), (9, 10), (10, 11), (11, 12), (12, 13)],
}
WARMUP_MM = 12  # dummy matmuls to ramp the PE clock before real data lands

_CACHE = {}


def _build_weights(Wc, bc):
    Weff = np.asarray(Wc, dtype=np.float32).sum(axis=0)  # [co, ci, dy, dx]
    beff = np.asarray(bc, dtype=np.float32).sum(axis=0)  # [co]
    S = np.zeros((MPAD, 3, MPAD), dtype=np.float32)  # 128 rows: even SDMA split
    for dx in range(3):
        for c_in in range(C):
            for c_out in range(C):
                for yo in range(40):
                    for dy in range(3):
                        S[c_in * 42 + yo + dy, dx, c_out * 40 + yo] = Weff[c_out, c_in, dy, dx]
    # bias rides the ones-row through the dx=0 (start) matmul
    for c_out in range(C):
        S[C * 42, 0, c_out * 40:(c_out + 1) * 40] = beff[c_out]
    return S.astype(np.float16)


def _build_program():
    import concourse.bass as bass
    import concourse.mybir as mybir
    import concourse.tile as tile
    from concourse import bacc

    nc = bacc.Bacc("TRN2", target_bir_lowering=False, debug=False)

    # XS carries only the 126 data rows: a 127-partition DMA cannot be split
    # evenly across the SDMA engines (127 is prime) and lands on ONE engine
    # at ~26GB/s (measured). The ones-row is a separate 1-descriptor DMA.
    XS = nc.dram_tensor("XS", [B_PER_CORE, KP - 1, NBLK, W], mybir.dt.float16, kind="ExternalInput")
    ONES = nc.dram_tensor("ONES", [1, NBLK, W], mybir.dt.float16, kind="ExternalInput")
    SMAT = nc.dram_tensor("SMAT", [MPAD, 3, MPAD], mybir.dt.float16, kind="ExternalInput")
    OUT = nc.dram_tensor("OUT", [B_PER_CORE, MP, NBLK, OW], mybir.dt.float16, kind="ExternalOutput")

    f32 = mybir.dt.float32
    f16 = mybir.dt.float16

    with tile.TileContext(nc) as tc:
        with (
            tc.tile_pool(name="consts", bufs=1) as consts,
            tc.tile_pool(name="xs", bufs=4) as xpool,
            tc.tile_pool(name="os", bufs=4) as opool,
            tc.tile_pool(name="ps", bufs=7, space=bass.MemorySpace.PSUM) as ppool,
        ):
            # one DMA, 128 descriptors of 768B, on the Act ring (the SP ring
            # belongs to the input stream)
            smat_t = consts.tile([MPAD, 3, MPAD], f16, tag="smat")
            nc.scalar.dma_start(out=smat_t[:], in_=SMAT.ap())

            # PE p-state warmup: the tensor clock ramps 0.65 -> 2.4GHz only
            # after ~3us of continuous work, so the first real matmuls would
            # run at half speed. Burn dummy matmuls on an uninitialized
            # scratch tile into a scratch PSUM bank while the input DMA is in
            # flight (start=stop=True, result never read; a later start=True
            # reset would clear any NaN garbage anyway -- this bank is unused).
            wsrc = consts.tile([MPAD, W], f16, tag="warmup_src")
            nc.vector.memset(wsrc[:], 1.0)
            wp = ppool.tile([MPAD, OW], f32, bufs=1)
            for _ in range(WARMUP_MM):
                nc.tensor.matmul(wp[:], wsrc[:, 0:MPAD], wsrc[:, 0:OW], start=True,
                                 stop=True, skip_group_check=True)

            # Phase A: queue the ENTIRE input stream up front, split across
            # BOTH HWDGE rings so the stream keeps ahead of compute (one ring
            # at ~250-330GB/s underruns the early consumer). All 4 images
            # stay resident in SBUF (~6.7MB). Each ring is FIFO: inputs go
            # first, output triggers for a ring are emitted only for images
            # whose data is fully queued before them.
            IN_ENG = {0: nc.sync, 1: nc.scalar, 2: nc.sync, 3: nc.scalar}
            xbs = []
            for img in range(B_PER_CORE):
                eng = IN_ENG[img]
                xb = xpool.tile([KP, NBLK, W], f16)
                eng.dma_start(out=xb[KP - 1:KP, :, :], in_=ONES.ap())
                for b0, b1 in IN_CHUNKS[img]:
                    eng.dma_start(out=xb[:KP - 1, b0:b1, :], in_=XS.ap()[img, :, b0:b1, :])
                xbs.append(xb)

            # Phase B: compute + copies; outputs use the ring opposite to the
            # one busy with that period's inputs (out0->Act after in3 queued,
            # out1->SP, out2->Act, img3's fine tail alternates both).
            OUT_ENG = {0: nc.scalar, 1: nc.sync, 2: nc.scalar}
            copy_idx = 0
            out_idx = 0
            for img in range(B_PER_CORE):
                xb = xbs[img]
                ot = opool.tile([MP, NBLK, OW], f16)
                for b0, b1 in OUT_CHUNKS[img]:
                    for b in range(b0, b1):
                        pt = ppool.tile([MPAD, OW], f32)
                        for dx in range(3):
                            nc.tensor.matmul(
                                pt[:],
                                smat_t[0:KP, dx, :],
                                xb[:, b, dx:dx + OW],
                                start=(dx == 0),
                                stop=(dx == 2),
                            )
                        # PSUM -> SBUF downcast (bias already in PSUM),
                        # alternating between the two PSUM-capable engines
                        if copy_idx % 2 == 0:
                            nc.scalar.copy(ot[:, b, :], pt[0:MP, :])
                        else:
                            nc.vector.tensor_copy(ot[:, b, :], pt[0:MP, :])
                        copy_idx += 1
                    if img in OUT_ENG:
                        eng = OUT_ENG[img]
                    else:
                        eng = nc.sync if out_idx % 2 == 0 else nc.scalar
                        out_idx += 1
                    eng.dma_start(out=OUT.ap()[img, :, b0:b1, :], in_=ot[:, b0:b1, :])

    nc.compile()
    return nc


def _get_nc():
    if "nc" not in _CACHE:
        _CACHE["nc"] = _build_program()
    return _CACHE["nc"]


def run_spmd(in_maps, **kwargs):
    from concourse.bass_utils import run_bass_kernel_spmd

    nc = _get_nc()
    return run_bass_kernel_spmd(nc, in_maps, list(range(N_CORES)), **kwargs)


def make_in_maps(X, Wc, bc):
    X = np.ascontiguousarray(np.asarray(X, dtype=np.float32))
    Sb = _build_weights(Wc, bc)

    # overlap-window shard: XP[core, img, c*42+yi, b, x] = X[4*core+img, c, y0(b)+yi, x]
    Xr = X.reshape(N_CORES, B_PER_CORE, C, H, W)
    XP = np.empty((N_CORES, B_PER_CORE, KP - 1, NBLK, W), dtype=np.float16)
    XPw = XP.reshape(N_CORES, B_PER_CORE, C, 42, NBLK, W)
    s = Xr.strides
    win = np.lib.stride_tricks.as_strided(
        Xr, shape=(N_CORES, B_PER_CORE, C, 12, 42, W),
        strides=(s[0], s[1], s[2], 40 * s[3], s[3], s[4]))
    XPw[:, :, :, :, 0:12, :] = win.transpose(0, 1, 2, 4, 3, 5)
    XPw[:, :, :, :, 12, :] = Xr[:, :, :, 470:512, :]
    ones = np.ones((1, NBLK, W), dtype=np.float16)

    return [
        {"XS": XP[i], "ONES": ones, "SMAT": Sb}
        for i in range(N_CORES)
    ]


def gather_output(res):
    """[core][img, (c,yo), b, x] -> [32, 3, 510, 510]"""
    OUTP = np.stack([res.results[i]["OUT"] for i in range(N_CORES)]).astype(np.float32)
    R = OUTP.reshape(N_CORES, B_PER_CORE, C, 40, NBLK, OW)
    O = np.empty((N_CORES, B_PER_CORE, C, OH, OW), dtype=np.float32)
    O[:, :, :, 0:480, :] = (
        R[:, :, :, :, 0:12, :].transpose(0, 1, 2, 4, 3, 5).reshape(N_CORES, B_PER_CORE, C, 480, OW)
    )
    O[:, :, :, 480:OH, :] = R[:, :, :, 10:40, 12, :]
    return O.reshape(N_CORES * B_PER_CORE, C, OH, OW)


def kernel(X, Wc, bc, linW, linb):
    res = run_spmd(make_in_maps(X, Wc, bc))
    return gather_output(res)


# revision 29
# speedup vs baseline: 1.0372x; 1.0372x over previous
"""Trainium2 Bass kernel for nn_Block_47545287967557 (dense_cnn).

The reference module, simplified:
  - dead avgpool->linear->relu path (result unused)
  - sum over K=4 conv branches == ONE 3x3 VALID conv with weights Wc.sum(0)
    and bias bc.sum(0):  O[b,co,y,x] = sum_{ci,dy,dx} Weff[co,ci,dy,dx] *
    X[b,ci,y+dy,x+dx] + beff[co]
  X: [32,3,512,512] fp32 -> O: [32,3,510,510] fp32.

Strategy: pure data-parallel over batch across 8 NeuronCores (4 images each).
Per core the conv runs on the tensor engine as block-banded matmuls:
  contraction K = (c_in, yi) packed into 126 partitions (42-row y window)
  plus a constant-ones row 126 that carries the bias (stationary row 126 of
  the dx=0 matrix holds beff, so PSUM comes out pre-biased), output
  M = (c_out, yo) packed into 120 partitions (+8 zero pad to 128 for FWL),
  moving N = 510 x positions; one matmul per dx shift (3, PSUM-accumulated).
  13 y-blocks per image (y0 = 0,40,...,440,470; the last overlaps rows
  470..479 with identical values).

Precision/bandwidth: X is cast to fp16 on the HOST and DMA'd as fp16 (the
matmul runs in fp16; this halves input HBM traffic). fp8 was measured (CPU
sim, exact seed-0 data): e4m3 rel err 4.0e-2, e3m4 2.97e-2 -- both over the
2e-2 gate, so fp16 it is (3.4e-4). The PSUM->SBUF downcast copy alternates
between the scalar and vector engines (a single engine would serialize at
~35us). Output is stored fp16 (host upcasts to fp32 while unsharding).

DMA: trn2 has two HWDGE rings, FIFO per ring (sync/SP and scalar/Act).
Inputs stream on the SP ring, outputs + consts on the Act ring so reads and
writes overlap. Outputs of earlier images must NOT use the SP ring: a
sequencer issues in program order, so an output trigger there would block
later input triggers behind the compute dependency. Only the last image's
output is split across both rings (4 chunks) to halve the drain tail.
The stationary matrices load as ONE DMA [127, 3*128] (127 x 768B
descriptors, ~2.4us) -- as 3+1 separate tensors they were 555 x 256B
descriptors taking ~10us, stalling the first image's dx=1/2 matmuls.
"""

import sys

sys.path.insert(0, "/opt/trn_rl_repo")

import numpy as np

N_CORES = 8
B_PER_CORE = 4
C = 3
H = W = 512
OH = OW = 510
NBLK = 13
KP = C * 42 + 1  # 126 contraction partitions + ones row (bias)
MP = C * 40      # 120 live output partitions
MPAD = 128       # stationary columns padded for FWL
IN_CHUNKS = {
    0: [(0, 2), (2, 7), (7, 13)],   # small first chunk -> compute starts early
    1: [(0, 7), (7, 13)],           # half-image chunks pace arrivals to the
    2: [(0, 7), (7, 13)],           # early (ramp-speed) consumption rate
    3: [(0, 13)],
}
OUT_CHUNKS = {
    0: [(0, 7), (7, 13)],
    1: [(0, 7), (7, 13)],
    2: [(0, 7), (7, 13)],
    3: [(0, 4), (4, 7), (7, 9), (9, 10), (10, 11), (11, 12), (12, 13)],
}
WARMUP_MM = 14  # dummy matmuls to ramp the PE clock before real data lands

_CACHE = {}


def _build_weights(Wc, bc):
    Weff = np.asarray(Wc, dtype=np.float32).sum(axis=0)  # [co, ci, dy, dx]
    beff = np.asarray(bc, dtype=np.float32).sum(axis=0)  # [co]
    S = np.zeros((MPAD, 3, MPAD), dtype=np.float32)  # 128 rows: even SDMA split
    for dx in range(3):
        for c_in in range(C):
            for c_out in range(C):
                for yo in range(40):
                    for dy in range(3):
                        S[c_in * 42 + yo + dy, dx, c_out * 40 + yo] = Weff[c_out, c_in, dy, dx]
    # bias rides the ones-row through the dx=0 (start) matmul
    for c_out in range(C):
        S[C * 42, 0, c_out * 40:(c_out + 1) * 40] = beff[c_out]
    return S.astype(np.float16)


def _build_program():
    import concourse.bass as bass
    import concourse.mybir as mybir
    import concourse.tile as tile
    from concourse import bacc

    nc = bacc.Bacc("TRN2", target_bir_lowering=False, debug=False)

    # XS carries only the 126 data rows: a 127-partition DMA cannot be split
    # evenly across the SDMA engines (127 is prime) and lands on ONE engine
    # at ~26GB/s (measured). The ones-row is a separate 1-descriptor DMA.
    XS = nc.dram_tensor("XS", [B_PER_CORE, KP - 1, NBLK, W], mybir.dt.float16, kind="ExternalInput")
    ONES = nc.dram_tensor("ONES", [1, NBLK, W], mybir.dt.float16, kind="ExternalInput")
    SMAT = nc.dram_tensor("SMAT", [MPAD, 3, MPAD], mybir.dt.float16, kind="ExternalInput")
    OUT = nc.dram_tensor("OUT", [B_PER_CORE, MP, NBLK, OW], mybir.dt.float16, kind="ExternalOutput")

    f32 = mybir.dt.float32
    f16 = mybir.dt.float16

    with tile.TileContext(nc) as tc:
        with (
            tc.tile_pool(name="consts", bufs=1) as consts,
            tc.tile_pool(name="xs", bufs=4) as xpool,
            tc.tile_pool(name="os", bufs=4) as opool,
            tc.tile_pool(name="ps", bufs=7, space=bass.MemorySpace.PSUM) as ppool,
        ):
            # one DMA, 128 descriptors of 768B, on the Act ring (the SP ring
            # belongs to the input stream)
            smat_t = consts.tile([MPAD, 3, MPAD], f16, tag="smat")
            nc.scalar.dma_start(out=smat_t[:], in_=SMAT.ap())

            # PE p-state warmup: the tensor clock ramps 0.65 -> 2.4GHz only
            # after ~3us of continuous work, so the first real matmuls would
            # run at half speed. Burn dummy matmuls on an uninitialized
            # scratch tile into a scratch PSUM bank while the input DMA is in
            # flight (start=stop=True, result never read; a later start=True
            # reset would clear any NaN garbage anyway -- this bank is unused).
            wsrc = consts.tile([MPAD, W], f16, tag="warmup_src")
            nc.vector.memset(wsrc[:], 1.0)
            wp = ppool.tile([MPAD, OW], f32, bufs=1)
            for _ in range(WARMUP_MM):
                nc.tensor.matmul(wp[:], wsrc[:, 0:MPAD], wsrc[:, 0:OW], start=True,
                                 stop=True, skip_group_check=True)

            # Phase A: queue the ENTIRE input stream on the SP ring up front.
            # HBM READS share ~360GB/s across both rings (measured: splitting
            # inputs over both halved each ring's read rate), so all reads
            # stay here and the Act ring gets the writes. All 4 images stay
            # resident in SBUF (~6.7MB).
            xbs = []
            for img in range(B_PER_CORE):
                xb = xpool.tile([KP, NBLK, W], f16)
                nc.sync.dma_start(out=xb[KP - 1:KP, :, :], in_=ONES.ap())
                for b0, b1 in IN_CHUNKS[img]:
                    nc.sync.dma_start(out=xb[:KP - 1, b0:b1, :], in_=XS.ap()[img, :, b0:b1, :])
                xbs.append(xb)

            # Phase B: compute + copies; outputs go on the Act ring while
            # inputs stream, alternating both rings for the last image's
            # fine-grained drain (the SP ring is free by then).
            copy_idx = 0
            out_idx = 0
            for img in range(B_PER_CORE):
                xb = xbs[img]
                ot = opool.tile([MP, NBLK, OW], f16)
                for b0, b1 in OUT_CHUNKS[img]:
                    for b in range(b0, b1):
                        pt = ppool.tile([MPAD, OW], f32)
                        for dx in range(3):
                            nc.tensor.matmul(
                                pt[:],
                                smat_t[0:KP, dx, :],
                                xb[:, b, dx:dx + OW],
                                start=(dx == 0),
                                stop=(dx == 2),
                            )
                        # PSUM -> SBUF downcast (bias already in PSUM),
                        # alternating between the two PSUM-capable engines
                        if copy_idx % 2 == 0:
                            nc.scalar.copy(ot[:, b, :], pt[0:MP, :])
                        else:
                            nc.vector.tensor_copy(ot[:, b, :], pt[0:MP, :])
                        copy_idx += 1
                    if img == B_PER_CORE - 1:
                        eng = nc.sync if out_idx % 2 == 0 else nc.scalar
                        out_idx += 1
                    else:
                        eng = nc.scalar
                    eng.dma_start(out=OUT.ap()[img, :, b0:b1, :], in_=ot[:, b0:b1, :])

    nc.compile()
    return nc


def _get_nc():
    if "nc" not in _CACHE:
        _CACHE["nc"] = _build_program()
    return _CACHE["nc"]


def run_spmd(in_maps, **kwargs):
    from concourse.bass_utils import run_bass_kernel_spmd

    nc = _get_nc()
    return run_bass_kernel_spmd(nc, in_maps, list(range(N_CORES)), **kwargs)


def make_in_maps(X, Wc, bc):
    X = np.ascontiguousarray(np.asarray(X, dtype=np.float32))
    Sb = _build_weights(Wc, bc)

    # overlap-window shard: XP[core, img, c*42+yi, b, x] = X[4*core+img, c, y0(b)+yi, x]
    Xr = X.reshape(N_CORES, B_PER_CORE, C, H, W)
    XP = np.empty((N_CORES, B_PER_CORE, KP - 1, NBLK, W), dtype=np.float16)
    XPw = XP.reshape(N_CORES, B_PER_CORE, C, 42, NBLK, W)
    s = Xr.strides
    win = np.lib.stride_tricks.as_strided(
        Xr, shape=(N_CORES, B_PER_CORE, C, 12, 42, W),
        strides=(s[0], s[1], s[2], 40 * s[3], s[3], s[4]))
    XPw[:, :, :, :, 0:12, :] = win.transpose(0, 1, 2, 4, 3, 5)
    XPw[:, :, :, :, 12, :] = Xr[:, :, :, 470:512, :]
    ones = np.ones((1, NBLK, W), dtype=np.float16)

    return [
        {"XS": XP[i], "ONES": ones, "SMAT": Sb}
        for i in range(N_CORES)
    ]


def gather_output(res):
    """[core][img, (c,yo), b, x] -> [32, 3, 510, 510]"""
    OUTP = np.stack([res.results[i]["OUT"] for i in range(N_CORES)]).astype(np.float32)
    R = OUTP.reshape(N_CORES, B_PER_CORE, C, 40, NBLK, OW)
    O = np.empty((N_CORES, B_PER_CORE, C, OH, OW), dtype=np.float32)
    O[:, :, :, 0:480, :] = (
        R[:, :, :, :, 0:12, :].transpose(0, 1, 2, 4, 3, 5).reshape(N_CORES, B_PER_CORE, C, 480, OW)
    )
    O[:, :, :, 480:OH, :] = R[:, :, :, 10:40, 12, :]
    return O.reshape(N_CORES * B_PER_CORE, C, OH, OW)


def kernel(X, Wc, bc, linW, linb):
    res = run_spmd(make_in_maps(X, Wc, bc))
    return gather_output(res)


# revision 40
# speedup vs baseline: 1.2312x; 1.1870x over previous
"""Trainium2 Bass kernel for nn_Block_47545287967557 (dense_cnn).

The reference module, simplified:
  - dead avgpool->linear->relu path (result unused)
  - sum over K=4 conv branches == ONE 3x3 VALID conv with weights Wc.sum(0)
    and bias bc.sum(0):  O[b,co,y,x] = sum_{ci,dy,dx} Weff[co,ci,dy,dx] *
    X[b,ci,y+dy,x+dx] + beff[co]
  X: [32,3,512,512] fp32 -> O: [32,3,510,510] fp32.

Strategy: pure data-parallel over batch across 8 NeuronCores (4 images each).
Per core the conv runs on the tensor engine as block-banded matmuls:
  contraction K = (c_in, yi) packed into 126 partitions (42-row y window)
  + ones row 126 carrying the bias (stationary row 126 of the dx=0 matrix
  holds beff so PSUM comes out pre-biased) + zero row 127; output
  M = (c_out, yo) packed into 120 partitions (+8 zero pad to 128), moving
  N = 510 x positions; one matmul per dx shift (3, PSUM-accumulated).
  13 y-blocks per image (y0 = 0,40,...,440,470; the last overlaps rows
  470..479 with identical values, deduped by the host on gather).

Precision: X is cast to fp16 on the HOST and DMA'd as fp16 (halves input
HBM traffic; the matmul ran in fp16 anyway). fp8 was measured on the exact
seed-0 data (CPU sim): e4m3 rel 4.0e-2, e3m4 2.97e-2 -- both over the 2e-2
gate; fp16 gives 3.4e-4. Output is stored fp16 (host upcasts on gather).

Schedule/DMA lessons baked in (all measured via NTFF profiles):
  - trn2 has two HWDGE rings, FIFO per issuing engine (sync/SP, scalar/Act).
    ALL input reads stream on the SP ring, queued up front for all 4 images
    (resident in SBUF, ~7MB); outputs + consts go on the Act ring. An
    output trigger on the SP ring would FIFO-block later input triggers.
    Only the last image's output drain alternates across both rings.
  - A DMA splits over the 16 SDMA engines (~23.5GB/s each) by partition
    groups: 128-partition transfers use all 16 (126 -> 14, 127 (prime) ->
    ONE engine at 26GB/s). Hence XS is padded to 128 rows.
  - The stationary set loads as ONE [128, 3*128] DMA (768B descriptors);
    as 3+1 tensors it was 555 x 256B descriptors taking ~10us.
  - Input chunks are 2-4 blocks: arrival then paces the early (clock-ramp
    speed) consumption and avoids underrun stalls that reset the ramp.
  - The PE clock ramps 0.65 -> 2.4GHz over ~10us of activity; 8 dummy
    warmup matmuls on a scratch tile start the ramp while the first input
    chunk is in flight (sized to end exactly when real data lands).
  - PSUM->SBUF downcast copies alternate scalar/vector (one engine would
    serialize at ~35us) and drain block PAIRS from 2-bank PSUM tiles.
"""

import sys

sys.path.insert(0, "/opt/trn_rl_repo")

import numpy as np

N_CORES = 8
B_PER_CORE = 4
C = 3
H = W = 512
OH = OW = 510
NBLK = 13
MPAD = 128       # stationary columns padded for FWL
KP = MPAD        # 126 data rows + ones row (bias) + zero pad row = 128:
                 # full 16-way SDMA split (126 rows use only 14 engines)
MP = C * 40      # 120 live output partitions
IN_CHUNKS = {
    0: [(0, 2), (2, 5), (5, 9), (9, 13)],  # small first chunk -> early start
    1: [(0, 3), (3, 6), (6, 10), (10, 13)],  # fine chunks pace arrivals to
    2: [(0, 3), (3, 6), (6, 10), (10, 13)],  # the early consumption rate
    3: [(0, 4), (4, 8), (8, 13)],
}
OUT_CHUNKS = {
    0: [(0, 7), (7, 13)],
    1: [(0, 7), (7, 13)],
    2: [(0, 7), (7, 13)],
    3: [(0, 4), (4, 8), (8, 10), (10, 12), (12, 13)],
}
WARMUP_MM = 8   # dummy matmuls to ramp the PE clock before real data lands

_CACHE = {}


def _build_weights(Wc, bc):
    Weff = np.asarray(Wc, dtype=np.float32).sum(axis=0)  # [co, ci, dy, dx]
    beff = np.asarray(bc, dtype=np.float32).sum(axis=0)  # [co]
    S = np.zeros((MPAD, 3, MPAD), dtype=np.float32)  # 128 rows: even SDMA split
    for dx in range(3):
        for c_in in range(C):
            for c_out in range(C):
                for yo in range(40):
                    for dy in range(3):
                        S[c_in * 42 + yo + dy, dx, c_out * 40 + yo] = Weff[c_out, c_in, dy, dx]
    # bias rides the ones-row through the dx=0 (start) matmul
    for c_out in range(C):
        S[C * 42, 0, c_out * 40:(c_out + 1) * 40] = beff[c_out]
    return S.astype(np.float16)


def _build_program():
    import concourse.bass as bass
    import concourse.mybir as mybir
    import concourse.tile as tile
    from concourse import bacc

    nc = bacc.Bacc("TRN2", target_bir_lowering=False, debug=False)

    # XS rows: 126 data + ones (bias row) + zero pad = 128 partitions. The
    # partition count must split evenly over the 16 SDMA engines (each tops
    # out at ~23.5GB/s): 127 rows (prime) land on ONE engine at 26GB/s,
    # 126 rows use only 14 engines; 128 uses all 16.
    XS = nc.dram_tensor("XS", [B_PER_CORE, KP, NBLK, W], mybir.dt.float16, kind="ExternalInput")
    SMAT = nc.dram_tensor("SMAT", [MPAD, 3, MPAD], mybir.dt.float16, kind="ExternalInput")
    OUT = nc.dram_tensor("OUT", [B_PER_CORE, MP, NBLK, OW], mybir.dt.float16, kind="ExternalOutput")

    f32 = mybir.dt.float32
    f16 = mybir.dt.float16

    with tile.TileContext(nc) as tc:
        with (
            tc.tile_pool(name="consts", bufs=1) as consts,
            tc.tile_pool(name="xs", bufs=4) as xpool,
            tc.tile_pool(name="os", bufs=4) as opool,
            tc.tile_pool(name="ps", bufs=3, space=bass.MemorySpace.PSUM) as ppool,
        ):
            # one DMA, 128 descriptors of 768B, on the Act ring (the SP ring
            # belongs to the input stream)
            smat_t = consts.tile([MPAD, 3, MPAD], f16, tag="smat")
            nc.scalar.dma_start(out=smat_t[:], in_=SMAT.ap())

            # PE p-state warmup: the tensor clock ramps 0.65 -> 2.4GHz only
            # after ~3us of continuous work, so the first real matmuls would
            # run at half speed. Burn dummy matmuls on an uninitialized
            # scratch tile into a scratch PSUM bank while the input DMA is in
            # flight (start=stop=True, result never read; a later start=True
            # reset would clear any NaN garbage anyway -- this bank is unused).
            wsrc = consts.tile([MPAD, W], f16, tag="warmup_src")
            nc.vector.memset(wsrc[:], 1.0)
            wp = ppool.tile([MPAD, OW], f32, bufs=2, tag="pt1")
            for _ in range(WARMUP_MM):
                nc.tensor.matmul(wp[:], wsrc[:, 0:MPAD], wsrc[:, 0:OW], start=True,
                                 stop=True, skip_group_check=True)

            # Phase A: queue the ENTIRE input stream on the SP ring up front.
            # HBM READS share ~360GB/s across both rings (measured: splitting
            # inputs over both halved each ring's read rate), so all reads
            # stay here and the Act ring gets the writes. All 4 images stay
            # resident in SBUF (~6.7MB).
            xbs = []
            for img in range(B_PER_CORE):
                xb = xpool.tile([KP, NBLK, W], f16)
                for b0, b1 in IN_CHUNKS[img]:
                    nc.sync.dma_start(out=xb[:, b0:b1, :], in_=XS.ap()[img, :, b0:b1, :])
                xbs.append(xb)

            # Phase B: compute + copies; outputs go on the Act ring while
            # inputs stream, alternating both rings for the last image's
            # fine-grained drain (the SP ring is free by then).
            copy_idx = 0
            out_idx = 0
            for img in range(B_PER_CORE):
                xb = xbs[img]
                ot = opool.tile([MP, NBLK, OW], f16)
                for b0, b1 in OUT_CHUNKS[img]:
                    # process blocks in PAIRS: one matmul group writes a
                    # 2-bank PSUM tile [128, 2, 512] and ONE copy drains both
                    # blocks -- halves matmul/copy instruction count and the
                    # cross-engine semaphore population (teardown clears every
                    # semaphore serially, ~9us measured with per-block ops)
                    b = b0
                    while b < b1:
                        nb = 2 if b + 2 <= b1 else 1
                        if nb == 2:
                            # two 1-block matmul groups into the two bank-
                            # halves of one PSUM tile, drained by ONE copy
                            pt = ppool.tile([MPAD, 2, W], f32)
                            for j in range(2):
                                for dx in range(3):
                                    nc.tensor.matmul(
                                        pt[:, j, 0:OW],
                                        smat_t[:, dx, :],
                                        xb[:, b + j, dx:dx + OW],
                                        start=(dx == 0),
                                        stop=(dx == 2),
                                        skip_group_check=True,
                                    )
                            src = pt[0:MP, :, 0:OW]
                        else:
                            pt = ppool.tile([MPAD, OW], f32, bufs=2, tag="pt1")
                            for dx in range(3):
                                nc.tensor.matmul(
                                    pt[:],
                                    smat_t[:, dx, :],
                                    xb[:, b, dx:dx + OW],
                                    start=(dx == 0),
                                    stop=(dx == 2),
                                )
                            src = pt[0:MP, :]
                        # PSUM -> SBUF downcast (bias already in PSUM),
                        # alternating between the two PSUM-capable engines
                        if copy_idx % 2 == 0:
                            nc.scalar.copy(ot[:, b:b + nb, :], src)
                        else:
                            nc.vector.tensor_copy(ot[:, b:b + nb, :], src)
                        copy_idx += 1
                        b += nb
                    if img == B_PER_CORE - 1:
                        eng = nc.sync if out_idx % 2 == 0 else nc.scalar
                        out_idx += 1
                    else:
                        eng = nc.scalar
                    eng.dma_start(out=OUT.ap()[img, :, b0:b1, :], in_=ot[:, b0:b1, :])

    nc.compile()
    return nc


def _get_nc():
    if "nc" not in _CACHE:
        _CACHE["nc"] = _build_program()
    return _CACHE["nc"]


def run_spmd(in_maps, **kwargs):
    from concourse.bass_utils import run_bass_kernel_spmd

    nc = _get_nc()
    return run_bass_kernel_spmd(nc, in_maps, list(range(N_CORES)), **kwargs)


def make_in_maps(X, Wc, bc):
    X = np.ascontiguousarray(np.asarray(X, dtype=np.float32))
    Sb = _build_weights(Wc, bc)

    # overlap-window shard: XP[core, img, c*42+yi, b, x] = X[4*core+img, c, y0(b)+yi, x]
    Xr = X.reshape(N_CORES, B_PER_CORE, C, H, W)
    XP = np.empty((N_CORES, B_PER_CORE, KP, NBLK, W), dtype=np.float16)
    XPw = XP[:, :, :C * 42].reshape(N_CORES, B_PER_CORE, C, 42, NBLK, W)
    s = Xr.strides
    win = np.lib.stride_tricks.as_strided(
        Xr, shape=(N_CORES, B_PER_CORE, C, 12, 42, W),
        strides=(s[0], s[1], s[2], 40 * s[3], s[3], s[4]))
    XPw[:, :, :, :, 0:12, :] = win.transpose(0, 1, 2, 4, 3, 5)
    XPw[:, :, :, :, 12, :] = Xr[:, :, :, 470:512, :]
    XP[:, :, C * 42, :, :] = np.float16(1.0)   # bias ones-row
    XP[:, :, C * 42 + 1, :, :] = np.float16(0.0)  # pad row (stationary row 127 = 0)

    return [
        {"XS": XP[i], "SMAT": Sb}
        for i in range(N_CORES)
    ]


def gather_output(res):
    """[core][img, (c,yo), b, x] -> [32, 3, 510, 510]"""
    OUTP = np.stack([res.results[i]["OUT"] for i in range(N_CORES)]).astype(np.float32)
    R = OUTP.reshape(N_CORES, B_PER_CORE, C, 40, NBLK, OW)
    O = np.empty((N_CORES, B_PER_CORE, C, OH, OW), dtype=np.float32)
    O[:, :, :, 0:480, :] = (
        R[:, :, :, :, 0:12, :].transpose(0, 1, 2, 4, 3, 5).reshape(N_CORES, B_PER_CORE, C, 480, OW)
    )
    O[:, :, :, 480:OH, :] = R[:, :, :, 10:40, 12, :]
    return O.reshape(N_CORES * B_PER_CORE, C, OH, OW)


def kernel(X, Wc, bc, linW, linb):
    res = run_spmd(make_in_maps(X, Wc, bc))
    return gather_output(res)


# revision 41
# speedup vs baseline: 1.2472x; 1.0131x over previous
"""Trainium2 Bass kernel for nn_Block_47545287967557 (dense_cnn).

The reference module, simplified:
  - dead avgpool->linear->relu path (result unused)
  - sum over K=4 conv branches == ONE 3x3 VALID conv with weights Wc.sum(0)
    and bias bc.sum(0):  O[b,co,y,x] = sum_{ci,dy,dx} Weff[co,ci,dy,dx] *
    X[b,ci,y+dy,x+dx] + beff[co]
  X: [32,3,512,512] fp32 -> O: [32,3,510,510] fp32.

Strategy: pure data-parallel over batch across 8 NeuronCores (4 images each).
Per core the conv runs on the tensor engine as block-banded matmuls:
  contraction K = (c_in, yi) packed into 126 partitions (42-row y window)
  + ones row 126 carrying the bias (stationary row 126 of the dx=0 matrix
  holds beff so PSUM comes out pre-biased) + zero row 127; output
  M = (c_out, yo) packed into 120 partitions (+8 zero pad to 128), moving
  N = 510 x positions; one matmul per dx shift (3, PSUM-accumulated).
  13 y-blocks per image (y0 = 0,40,...,440,470; the last overlaps rows
  470..479 with identical values, deduped by the host on gather).

Precision: X is cast to fp16 on the HOST and DMA'd as fp16 (halves input
HBM traffic; the matmul ran in fp16 anyway). fp8 was measured on the exact
seed-0 data (CPU sim): e4m3 rel 4.0e-2, e3m4 2.97e-2 -- both over the 2e-2
gate; fp16 gives 3.4e-4. Output is stored fp16 (host upcasts on gather).

Schedule/DMA lessons baked in (all measured via NTFF profiles):
  - trn2 has two HWDGE rings, FIFO per issuing engine (sync/SP, scalar/Act).
    ALL input reads stream on the SP ring, queued up front for all 4 images
    (resident in SBUF, ~7MB); outputs + consts go on the Act ring. An
    output trigger on the SP ring would FIFO-block later input triggers.
    Only the last image's output drain alternates across both rings.
  - A DMA splits over the 16 SDMA engines (~23.5GB/s each) by partition
    groups: 128-partition transfers use all 16 (126 -> 14, 127 (prime) ->
    ONE engine at 26GB/s). Hence XS is padded to 128 rows.
  - The stationary set loads as ONE [128, 3*128] DMA (768B descriptors);
    as 3+1 tensors it was 555 x 256B descriptors taking ~10us.
  - Input chunks are 2-4 blocks: arrival then paces the early (clock-ramp
    speed) consumption and avoids underrun stalls that reset the ramp.
  - The PE clock ramps 0.65 -> 2.4GHz over ~10us of activity; 8 dummy
    warmup matmuls on a scratch tile start the ramp while the first input
    chunk is in flight (sized to end exactly when real data lands).
  - PSUM->SBUF downcast copies alternate scalar/vector (one engine would
    serialize at ~35us) and drain block PAIRS from 2-bank PSUM tiles.
"""

import sys

sys.path.insert(0, "/opt/trn_rl_repo")

import numpy as np

N_CORES = 8
B_PER_CORE = 4
C = 3
H = W = 512
OH = OW = 510
NBLK = 13
MPAD = 128       # stationary columns padded for FWL
KP = MPAD        # 126 data rows + ones row (bias) + zero pad row = 128:
                 # full 16-way SDMA split (126 rows use only 14 engines)
MP = C * 40      # 120 live output partitions
IN_CHUNKS = {
    0: [(0, 2), (2, 5), (5, 9), (9, 13)],  # small first chunk -> early start
    1: [(0, 3), (3, 6), (6, 10), (10, 13)],  # fine chunks pace arrivals to
    2: [(0, 3), (3, 6), (6, 10), (10, 13)],  # the early consumption rate
    3: [(0, 4), (4, 8), (8, 13)],
}
OUT_CHUNKS = {
    0: [(0, 7), (7, 13)],
    1: [(0, 7), (7, 13)],
    2: [(0, 7), (7, 13)],
    3: [(0, 4), (4, 8), (8, 10), (10, 12), (12, 13)],
}
WARMUP_MM = 8   # dummy matmuls to ramp the PE clock before real data lands

_CACHE = {}


def _build_weights(Wc, bc):
    Weff = np.asarray(Wc, dtype=np.float32).sum(axis=0)  # [co, ci, dy, dx]
    beff = np.asarray(bc, dtype=np.float32).sum(axis=0)  # [co]
    S = np.zeros((MPAD, 3, MPAD), dtype=np.float32)  # 128 rows: even SDMA split
    for dx in range(3):
        for c_in in range(C):
            for c_out in range(C):
                for yo in range(40):
                    for dy in range(3):
                        S[c_in * 42 + yo + dy, dx, c_out * 40 + yo] = Weff[c_out, c_in, dy, dx]
    # bias rides the ones-row through the dx=0 (start) matmul
    for c_out in range(C):
        S[C * 42, 0, c_out * 40:(c_out + 1) * 40] = beff[c_out]
    return S.astype(np.float16)


def _build_program():
    import concourse.bass as bass
    import concourse.mybir as mybir
    import concourse.tile as tile
    from concourse import bacc

    nc = bacc.Bacc("TRN2", target_bir_lowering=False, debug=False)

    # XS rows: 126 data + ones (bias row) + zero pad = 128 partitions. The
    # partition count must split evenly over the 16 SDMA engines (each tops
    # out at ~23.5GB/s): 127 rows (prime) land on ONE engine at 26GB/s,
    # 126 rows use only 14 engines; 128 uses all 16.
    XS = nc.dram_tensor("XS", [B_PER_CORE, KP, NBLK, W], mybir.dt.float16, kind="ExternalInput")
    SMAT = nc.dram_tensor("SMAT", [MPAD, 3, MPAD], mybir.dt.float16, kind="ExternalInput")
    OUT = nc.dram_tensor("OUT", [B_PER_CORE, MP, NBLK, OW], mybir.dt.float16, kind="ExternalOutput")

    f32 = mybir.dt.float32
    f16 = mybir.dt.float16

    with tile.TileContext(nc) as tc:
        with (
            tc.tile_pool(name="consts", bufs=1) as consts,
            tc.tile_pool(name="xs", bufs=4) as xpool,
            tc.tile_pool(name="os", bufs=4) as opool,
            tc.tile_pool(name="ps", bufs=3, space=bass.MemorySpace.PSUM) as ppool,
        ):
            # one DMA, 128 descriptors of 768B, on the Act ring (the SP ring
            # belongs to the input stream)
            smat_t = consts.tile([MPAD, 3, MPAD], f16, tag="smat")
            nc.scalar.dma_start(out=smat_t[:], in_=SMAT.ap())

            # PE p-state warmup: the tensor clock ramps 0.65 -> 2.4GHz only
            # after ~3us of continuous work, so the first real matmuls would
            # run at half speed. Burn dummy matmuls on an uninitialized
            # scratch tile into a scratch PSUM bank while the input DMA is in
            # flight (start=stop=True, result never read; a later start=True
            # reset would clear any NaN garbage anyway -- this bank is unused).
            wsrc = consts.tile([MPAD, W], f16, tag="warmup_src")
            nc.vector.memset(wsrc[:], 1.0)
            wp = ppool.tile([MPAD, OW], f32, bufs=2, tag="pt1")
            for _ in range(WARMUP_MM):
                nc.tensor.matmul(wp[:], wsrc[:, 0:MPAD], wsrc[:, 0:OW], start=True,
                                 stop=True, skip_group_check=True)

            # Phase A: queue the ENTIRE input stream on the SP ring up front.
            # HBM READS share ~360GB/s across both rings (measured: splitting
            # inputs over both halved each ring's read rate), so all reads
            # stay here and the Act ring gets the writes. All 4 images stay
            # resident in SBUF (~6.7MB).
            xbs = []
            for img in range(B_PER_CORE):
                xb = xpool.tile([KP, NBLK, W], f16)
                for b0, b1 in IN_CHUNKS[img]:
                    nc.sync.dma_start(out=xb[:, b0:b1, :], in_=XS.ap()[img, :, b0:b1, :])
                xbs.append(xb)

            # Phase B: compute + copies; outputs go on the Act ring while
            # inputs stream, alternating both rings for the last image's
            # fine-grained drain (the SP ring is free by then).
            copy_idx = 0
            out_idx = 0
            for img in range(B_PER_CORE):
                xb = xbs[img]
                ot = opool.tile([MP, NBLK, OW], f16)
                for b0, b1 in OUT_CHUNKS[img]:
                    # process blocks in PAIRS: one matmul group writes a
                    # 2-bank PSUM tile [128, 2, 512] and ONE copy drains both
                    # blocks -- halves matmul/copy instruction count and the
                    # cross-engine semaphore population (teardown clears every
                    # semaphore serially, ~9us measured with per-block ops)
                    b = b0
                    while b < b1:
                        nb = 2 if b + 2 <= b1 else 1
                        if nb == 2:
                            # two 1-block matmul groups into the two bank-
                            # halves of one PSUM tile, drained by ONE copy
                            pt = ppool.tile([MPAD, 2, W], f32)
                            for j in range(2):
                                for dx in range(3):
                                    nc.tensor.matmul(
                                        pt[:, j, 0:OW],
                                        smat_t[:, dx, :],
                                        xb[:, b + j, dx:dx + OW],
                                        start=(dx == 0),
                                        stop=(dx == 2),
                                        skip_group_check=True,
                                    )
                            src = pt[0:MP, :, 0:OW]
                        else:
                            pt = ppool.tile([MPAD, OW], f32, bufs=2, tag="pt1")
                            for dx in range(3):
                                nc.tensor.matmul(
                                    pt[:],
                                    smat_t[:, dx, :],
                                    xb[:, b, dx:dx + OW],
                                    start=(dx == 0),
                                    stop=(dx == 2),
                                )
                            src = pt[0:MP, :]
                        # PSUM -> SBUF downcast (bias already in PSUM),
                        # alternating between the two PSUM-capable engines
                        if copy_idx % 2 == 0:
                            nc.scalar.copy(ot[:, b:b + nb, :], src)
                        else:
                            nc.vector.tensor_copy(ot[:, b:b + nb, :], src)
                        copy_idx += 1
                        b += nb
                    if img == B_PER_CORE - 1:
                        eng = nc.sync if out_idx % 2 == 0 else nc.scalar
                        out_idx += 1
                    elif img == 0:
                        # img0's output is ready (~15us) while inputs still
                        # stream; queuing it on the SP ring FIFO-defers its
                        # descriptors until the input stream finishes, so
                        # inputs keep the whole shared SDMA budget
                        eng = nc.sync
                    else:
                        eng = nc.scalar
                    eng.dma_start(out=OUT.ap()[img, :, b0:b1, :], in_=ot[:, b0:b1, :])

    nc.compile()
    return nc


def _get_nc():
    if "nc" not in _CACHE:
        _CACHE["nc"] = _build_program()
    return _CACHE["nc"]


def run_spmd(in_maps, **kwargs):
    from concourse.bass_utils import run_bass_kernel_spmd

    nc = _get_nc()
    return run_bass_kernel_spmd(nc, in_maps, list(range(N_CORES)), **kwargs)


def make_in_maps(X, Wc, bc):
    X = np.ascontiguousarray(np.asarray(X, dtype=np.float32))
    Sb = _build_weights(Wc, bc)

    # overlap-window shard: XP[core, img, c*42+yi, b, x] = X[4*core+img, c, y0(b)+yi, x]
    Xr = X.reshape(N_CORES, B_PER_CORE, C, H, W)
    XP = np.empty((N_CORES, B_PER_CORE, KP, NBLK, W), dtype=np.float16)
    XPw = XP[:, :, :C * 42].reshape(N_CORES, B_PER_CORE, C, 42, NBLK, W)
    s = Xr.strides
    win = np.lib.stride_tricks.as_strided(
        Xr, shape=(N_CORES, B_PER_CORE, C, 12, 42, W),
        strides=(s[0], s[1], s[2], 40 * s[3], s[3], s[4]))
    XPw[:, :, :, :, 0:12, :] = win.transpose(0, 1, 2, 4, 3, 5)
    XPw[:, :, :, :, 12, :] = Xr[:, :, :, 470:512, :]
    XP[:, :, C * 42, :, :] = np.float16(1.0)   # bias ones-row
    XP[:, :, C * 42 + 1, :, :] = np.float16(0.0)  # pad row (stationary row 127 = 0)

    return [
        {"XS": XP[i], "SMAT": Sb}
        for i in range(N_CORES)
    ]


def gather_output(res):
    """[core][img, (c,yo), b, x] -> [32, 3, 510, 510]"""
    OUTP = np.stack([res.results[i]["OUT"] for i in range(N_CORES)]).astype(np.float32)
    R = OUTP.reshape(N_CORES, B_PER_CORE, C, 40, NBLK, OW)
    O = np.empty((N_CORES, B_PER_CORE, C, OH, OW), dtype=np.float32)
    O[:, :, :, 0:480, :] = (
        R[:, :, :, :, 0:12, :].transpose(0, 1, 2, 4, 3, 5).reshape(N_CORES, B_PER_CORE, C, 480, OW)
    )
    O[:, :, :, 480:OH, :] = R[:, :, :, 10:40, 12, :]
    return O.reshape(N_CORES * B_PER_CORE, C, OH, OW)


def kernel(X, Wc, bc, linW, linb):
    res = run_spmd(make_in_maps(X, Wc, bc))
    return gather_output(res)
